# revision 3
# baseline (speedup 1.0000x reference)
"""conv_downsample_2d on 8 TRN2 cores: FIR anti-alias + 3x3 stride-2 conv.

Math: the reference is a 6x6 stride-2 conv with kernel
K6 = w (*) outer(k,k)/64, k = (1,3,3,1).  Since (1,3,3,1) = (1,1)^(*3),
the ROW factor is applied to the DATA as 3 chained 2-tap box passes on
the VectorEngine (row-shifted adds keep operands 4B-aligned -> 2x_1P
mode, ~246 G elem/s; a column-shifted add would drop to 1x); the COLUMN
factor (and the 1/64) folds into the PE weights:
    K36[o,c,p,v] = sum_q w[o,c,p,q] k[v-q] / 64        (3 x 6 taps)
    y[o,i,j] = sum_{c,p,v} K36[o,c,p,v] * z[c, 2i+p, 2j+v] + b[o]
where z = 3x row-box of x zero-padded by 2.

Mapping: pure batch data-parallel, one image per NeuronCore.  Partition
= (row-band g in 0..3)*32 + channel; band g owns output rows
[96g, 96(g+1)) and a block-diagonal [128,128] bf16 weight per tap
computes all 4 bands in one matmul (N=384 output cols, fp32 PSUM).
Per strip of R=6 output rows per band, the x rows arrive as ONE
contiguous cast-DMA descriptor per partition (f32 HBM -> bf16 SBUF,
SWDGE); v-passes run on unpadded 768-wide tiles and only the final z
tile carries the 4 zero pad columns.  Output is stored bf16 and upcast
on the host (rel-err budget 2e-2 >> bf16 noise; measured 3.3e-3).

Pipeline: xb is triple-buffered with loads issued TWO strips ahead, so
the strip s+1 box chain runs on the DVE entirely under strip s's
matmuls; a 4-row SBUF copy-forward carries the strip halo instead of
re-reading HBM.  Strips 0/1 are processed in row-halves to cut the
startup critical path (load + 3 serial DVE passes), and the last strip
evicts/stores per half to shorten the tail.
"""

import numpy as np

N_BATCH = 8
C_IN = 32
C_OUT = 32
H = W = 768
HO = WO = 384
NCORES = 8

G = 4              # row bands (partition groups)
BH = HO // G       # 96 output rows per band
R = 6              # output rows per band per strip
NSTRIPS = BH // R  # 16
XR = 2 * R + 4     # 16 x rows per strip (incl 4-row halo)
ZR = 2 * R + 1     # 13 z rows per strip
NTAP = 18          # 3 rows x 6 cols
ZW = 772           # z tile width: x cols -2..769

_CACHE = {}
PROFILE = False
LAST_RESULTS = None


def _host_tensors(w: np.ndarray, b: np.ndarray):
    """Block-diag per-tap weights [NTAP,128,128] bf16 and bias [128,1] f32."""
    k = np.array([1.0, 3.0, 3.0, 1.0], np.float64)
    w64 = w.astype(np.float64)
    K36 = np.zeros((C_OUT, C_IN, 3, 6), np.float64)
    for q in range(3):
        for a in range(4):
            K36[:, :, :, q + a] += w64[:, :, :, q] * (k[a] / 64.0)
    K36 = K36.astype(np.float32)
    WT = np.zeros((NTAP, 128, 128), np.float32)
    blk = K36.transpose(2, 3, 1, 0).reshape(NTAP, C_IN, C_OUT)
    for g in range(G):
        WT[:, g * 32 : g * 32 + 32, g * 32 : g * 32 + 32] = blk
    # store in SBUF layout [k, t, m] so the weight DMA is contiguous
    # (the AP-rearrange form costs 2304 strided 256B descriptors that
    # stall the first matmul ~30us behind the big x cast-loads)
    WT = np.ascontiguousarray(WT.transpose(1, 0, 2))
    BIAS = np.tile(b.astype(np.float32), G).reshape(128, 1)
    return WT, BIAS


def _build_program():
    from contextlib import ExitStack

    import concourse.bacc as bacc
    import concourse.tile as tile
    from concourse import mybir

    f32 = mybir.dt.float32
    bf16 = mybir.dt.bfloat16

    nc = bacc.Bacc(
        "TRN2", target_bir_lowering=False, debug=False, num_devices=NCORES
    )
    x_d = nc.dram_tensor("x", [C_IN, H, W], f32, kind="ExternalInput").ap()
    wt_d = nc.dram_tensor("wt", [128, NTAP, 128], bf16, kind="ExternalInput").ap()
    bias_d = nc.dram_tensor("bias", [128, 1], f32, kind="ExternalInput").ap()
    y_d = nc.dram_tensor("y", [C_OUT, HO, WO], bf16, kind="ExternalOutput").ap()

    with tile.TileContext(nc) as tc, ExitStack() as ctx:
        wpool = ctx.enter_context(tc.tile_pool(name="wpool", bufs=1))
        xpool = ctx.enter_context(tc.tile_pool(name="xpool", bufs=1))
        opool = ctx.enter_context(tc.tile_pool(name="opool", bufs=2))
        ppool = ctx.enter_context(tc.tile_pool(name="ppool", bufs=8, space="PSUM"))

        wt_t = wpool.tile([128, NTAP, 128], bf16)
        nc.sync.dma_start(wt_t[:], wt_d[:])
        bias_t = wpool.tile([128, 1], f32)
        nc.sync.dma_start(bias_t[:], bias_d[:])

        xbs, zs = [], []
        for i in range(3):
            xb = xpool.tile([128, XR, W], bf16, tag=f"xb{i}", name=f"xb{i}")
            xbs.append(xb)
        for i in range(2):
            z = xpool.tile([128, ZR, ZW], bf16, tag=f"z{i}", name=f"z{i}")
            # pad cols 0,1,770,771: zeroed once; v3 writes only [2:770]
            nc.vector.memset(z[:, :, 0:2], 0.0)
            nc.vector.memset(z[:, :, ZW - 2 : ZW], 0.0)
            zs.append(z)
        d1 = xpool.tile([128, XR - 1, W], bf16, tag="d1")
        d2 = xpool.tile([128, XR - 2, W], bf16, tag="d2")

        def issue_loads(s, rows=None):
            """Strip s: cast-DMA x rows into xb (one descriptor/partition)."""
            xb = xbs[s % 3]
            r0, r1 = rows if rows else ((0 if s == 0 else 4), XR)
            for g in range(G):
                xs0 = 192 * g + 2 * R * s - 2  # x row of tile row 0
                r_lo = max(r0, -xs0)
                r_hi = min(r1, H - xs0)
                if r_lo > r0:
                    nc.vector.memset(xb[32 * g : 32 * (g + 1), r0:r_lo, :], 0.0)
                if r_hi < r1:
                    nc.vector.memset(xb[32 * g : 32 * (g + 1), r_hi:r1, :], 0.0)
                nc.gpsimd.dma_start(
                    xb[32 * g : 32 * (g + 1), r_lo:r_hi, :],
                    x_d[:, xs0 + r_lo : xs0 + r_hi, :],
                )

        def issue_copy_forward(s):
            """xb(s)[0:4] = xb(s-1)[2R:2R+4] (bf16 copy, 4x mode)."""
            nc.vector.tensor_copy(
                xbs[s % 3][:, 0:4, :], xbs[(s - 1) % 3][:, XR - 4 : XR, :]
            )

        def issue_boxes(s):
            xb, z = xbs[s % 3], zs[s % 2]
            nc.vector.tensor_add(d1[:, :, :], xb[:, 0 : XR - 1, :], xb[:, 1:XR, :])
            nc.vector.tensor_add(
                d2[:, :, :], d1[:, 0 : XR - 2, :], d1[:, 1 : XR - 1, :]
            )
            nc.vector.tensor_add(
                z[:, :, 2 : W + 2], d2[:, 0:ZR, :], d2[:, 1 : ZR + 1, :]
            )

        # strip 0 is loaded/boxed/matmul'd in two row-halves so the first
        # matmuls start ~20us earlier (the full-strip chain is the startup
        # critical path: 2-strip load @ line rate + 3 serial DVE passes).
        RH = 10  # xb row where strip 0 is split (z rows [0:7) / [7:ZR))
        issue_loads(0, rows=(0, RH))
        issue_loads(0, rows=(RH, XR))
        issue_loads(1)
        xb0, z0 = xbs[0], zs[0]
        nc.vector.tensor_add(d1[:, 0 : RH - 1, :], xb0[:, 0 : RH - 1, :], xb0[:, 1:RH, :])
        nc.vector.tensor_add(d2[:, 0 : RH - 2, :], d1[:, 0 : RH - 2, :], d1[:, 1 : RH - 1, :])
        nc.vector.tensor_add(
            z0[:, 0 : RH - 3, 2 : W + 2], d2[:, 0 : RH - 3, :], d2[:, 1 : RH - 2, :]
        )
        nc.vector.tensor_add(d1[:, RH - 1 : XR - 1, :], xb0[:, RH - 1 : XR - 1, :], xb0[:, RH:XR, :])
        nc.vector.tensor_add(d2[:, RH - 2 : XR - 2, :], d1[:, RH - 2 : XR - 2, :], d1[:, RH - 1 : XR - 1, :])
        nc.vector.tensor_add(
            z0[:, RH - 3 : ZR, 2 : W + 2], d2[:, RH - 3 : ZR, :], d2[:, RH - 2 : ZR + 1, :]
        )
        def issue_boxes_halved(s):
            """Like issue_boxes but in two row-chunks so z lands incrementally."""
            xb, z = xbs[s % 3], zs[s % 2]
            nc.vector.tensor_add(
                d1[:, 0 : RH - 1, :], xb[:, 0 : RH - 1, :], xb[:, 1:RH, :]
            )
            nc.vector.tensor_add(
                d2[:, 0 : RH - 2, :], d1[:, 0 : RH - 2, :], d1[:, 1 : RH - 1, :]
            )
            nc.vector.tensor_add(
                z[:, 0 : RH - 3, 2 : W + 2], d2[:, 0 : RH - 3, :], d2[:, 1 : RH - 2, :]
            )
            nc.vector.tensor_add(
                d1[:, RH - 1 : XR - 1, :], xb[:, RH - 1 : XR - 1, :], xb[:, RH:XR, :]
            )
            nc.vector.tensor_add(
                d2[:, RH - 2 : XR - 2, :], d1[:, RH - 2 : XR - 2, :], d1[:, RH - 1 : XR - 1, :]
            )
            nc.vector.tensor_add(
                z[:, RH - 3 : ZR, 2 : W + 2], d2[:, RH - 3 : ZR, :], d2[:, RH - 2 : ZR + 1, :]
            )

        for s in range(NSTRIPS):
            if s + 1 < NSTRIPS:
                issue_copy_forward(s + 1)
                if s + 1 == 1:
                    issue_boxes_halved(s + 1)
                else:
                    issue_boxes(s + 1)
            if s + 2 < NSTRIPS:
                issue_loads(s + 2)
            z = zs[s % 2]
            ot = opool.tile([128, R, WO], bf16)
            pts = [
                ppool.tile([128, 1, WO], mybir.dt.float32, tag="pt", name="pt")
                for _ in range(R)
            ]
            halved_mm = s in (0, 1, NSTRIPS - 1)
            halves = ((range(0, 3), range(3, R)) if halved_mm else (range(R),))
            for rr in halves:
                for t in range(NTAP):
                    p, v = divmod(t, 6)
                    lhsT = wt_t[:, t, :]
                    for r in rr:
                        rhs = z[:, 2 * r + p, v : v + 2 * WO - 1 : 2]
                        nc.tensor.matmul(
                            pts[r][:], lhsT, rhs,
                            start=(t == 0), stop=(t == NTAP - 1),
                        )
                if s == NSTRIPS - 1:
                    # drain the tail: evict+store each half as soon as it is done
                    for r in rr:
                        nc.scalar.activation(
                            ot[:, r, :], pts[r][:],
                            mybir.ActivationFunctionType.Identity, bias=bias_t[:],
                        )
                    ro, rn = rr[0], len(rr)
                    for g in range(G):
                        nc.sync.dma_start(
                            y_d[:, 96 * g + R * s + ro : 96 * g + R * s + ro + rn, :],
                            ot[32 * g : 32 * (g + 1), ro : ro + rn, :],
                        )
            if s != NSTRIPS - 1:
                for r, pt in enumerate(pts):
                    nc.scalar.activation(
                        ot[:, r, :], pt[:],
                        mybir.ActivationFunctionType.Identity, bias=bias_t[:],
                    )
                for g in range(G):
                    nc.sync.dma_start(
                        y_d[:, 96 * g + R * s : 96 * g + R * (s + 1), :],
                        ot[32 * g : 32 * (g + 1), :, :],
                    )

    nc.compile()
    return nc


def kernel(x: np.ndarray, w: np.ndarray, b: np.ndarray) -> np.ndarray:
    global LAST_RESULTS
    import ml_dtypes
    from concourse.bass_utils import run_bass_kernel_spmd

    x = np.ascontiguousarray(x, np.float32)
    WT, BIAS = _host_tensors(np.asarray(w, np.float32), np.asarray(b, np.float32))
    WTb = WT.astype(ml_dtypes.bfloat16)

    if "nc" not in _CACHE:
        _CACHE["nc"] = _build_program()
    nc = _CACHE["nc"]

    in_maps = [{"x": x[n], "wt": WTb, "bias": BIAS} for n in range(N_BATCH)]
    res = run_bass_kernel_spmd(nc, in_maps, list(range(NCORES)), trace=PROFILE)
    LAST_RESULTS = res
    out = np.stack([res.results[n]["y"] for n in range(N_BATCH)], axis=0)
    return out.astype(np.float32)
